# revision 23
# baseline (speedup 1.0000x reference)
"""4-bit column-block-quantized linear (ColBlockQuantizedLinear) on 8 Trainium2 NeuronCores.

Reference computation:
    w[n, k] = (nibble(quant_weight)[n, k] - zeros[n]) * scales[n]     n<11008, k<4096
    out[b, s, n] = sum_k inp[b, s, k] * w[n, k]                        inp: [4, 2048, 4096] f32

Strategy (column-parallel, per sharding hint):
  - Shard out_features N=11008 = 8*1376 across 8 cores; replicate inp.
  - fp8 double-pumped matmul (MatmulPerfMode.DoubleRow, 2x the bf16 PE rate):
    both operands are float8e4 (e4m3). Host ships activations rounded to e4m3
    and weights expanded to CENTERED nibbles (q - 7.5), which are exact in
    e4m3 (values +-0.5 .. +-7.5).
  - Centering is the accuracy trick: the fp8 rounding error of the
    activations couples to the matmul weights, so using (q - 7.5) instead of
    raw q (RMS 4.6 vs 8.8) cuts the error ~1.9x. The 7.5 shift is folded
    back exactly at eviction through the f64-accurate host row-sums:
        out = psum * s[n] + s[n]*(7.5 - z[n]) * rowsum[m]
    Measured l2 rel err ~1.7e-2 (vs 3.2e-2 uncentered).
  - K = 4096 = 16 pairs x (2 planes x 128); DoubleRow contracts both planes
    of a pair per instruction: lhsT = x8[128, 2, 128m], moving =
    w8[128, 2, 512n] (fp8 moving free dim max 1024; out = 512 f32 = exactly
    one PSUM bank). start=True marks the whole 2KB PSUM bank pending-zero,
    so only bank-first chunks may issue it.
  - Host pre-expands packed nibbles to centered fp8 weights (no on-chip
    unpack) and ships activations pre-rounded to e4m3: DMA totals ~41MB/core,
    well under the ~360GB/s budget for the ~600us PE-bound runtime.
  - A 120-instruction burst of narrow dummy matmuls warms the PE (HAM
    un-throttle to 8/8) while the DMA stream builds a head start; both the
    burst length and its ~8us duration are load-bearing (shorter warmups
    leave the clock throttled / the PE stalling on the DMA ramp).
  - Eviction is chunk-wise (ACT copy, VE scale, VE rowsum-correct, DMA out
    per 512-col bank) so the tail pipelines.
  - Host concatenates per-core outputs along N.
"""

import sys

for _p in ("/opt/trn_rl_repo", "/opt/pypackages"):
    if _p not in sys.path:
        sys.path.append(_p)

import numpy as np
import ml_dtypes

import concourse.bass as bass
import concourse.mybir as mybir
import concourse.tile as tile
from concourse import bacc

# Problem constants (hardcoded per harness contract)
B, S, K = 4, 2048, 4096
M = B * S                  # 8192 tokens
N = 11008                  # out features
NCORES = 8
NPC = N // NCORES          # per-core out features (1376)
P = 128
KPAIRS = K // (2 * P)      # 16 pairs of k-planes (256 k each)
CENTER = 7.5               # nibble centering; q - 7.5 is exact in e4m3


def _nchunks(npc, cw=256):
    return [(i, min(cw, npc - i)) for i in range(0, npc, cw)]


def build_nc(m=M, npc=NPC, mg=1024, warmup=120, cw=512):
    """Build the per-core Bass program. m tokens, npc out cols, mg tokens per
    m-group (DMA granule), cw psum chunk width (512 f32 = one PSUM bank;
    DoubleRow moving free = 2*cw <= 1024)."""
    ngroups = m // mg
    mbs = mg // P              # m-blocks per group
    chunks = _nchunks(npc, cw)
    f8 = mybir.dt.float8e4

    nc = bacc.Bacc("TRN2", target_bir_lowering=False, debug=False)
    x8_d = nc.dram_tensor("x8", [KPAIRS, P, 2, m], f8, kind="ExternalInput")
    w8_d = nc.dram_tensor("w8", [KPAIRS, P, 2, npc], f8, kind="ExternalInput")
    s_d = nc.dram_tensor("s32", [P, npc], mybir.dt.float32, kind="ExternalInput")
    cb_d = nc.dram_tensor("cb32", [P, npc], mybir.dt.float32, kind="ExternalInput")
    rs_d = nc.dram_tensor("rs", [P, m // P], mybir.dt.float32, kind="ExternalInput")
    out_d = nc.dram_tensor("out", [m, npc], mybir.dt.float32, kind="ExternalOutput")

    with tile.TileContext(nc) as tc:
        with (
            tc.tile_pool(name="const", bufs=1) as const_pool,
            tc.tile_pool(name="w", bufs=1) as w_pool,
            tc.tile_pool(name="x", bufs=3) as x_pool,
            tc.tile_pool(name="o", bufs=3) as o_pool,
            tc.tile_pool(name="ps", bufs=2, space="PSUM") as ps_pool,
            tc.tile_pool(name="wps", bufs=1, space="PSUM") as warm_ps_pool,
        ):
            s32t = const_pool.tile([P, npc], mybir.dt.float32, tag="s32t")
            cb32t = const_pool.tile([P, npc], mybir.dt.float32, tag="cb32t")
            rs_t = const_pool.tile([P, m // P], mybir.dt.float32, tag="rs_t")
            # PE warmup: flip the HAM clock gate to 8/8 while DMAs run. The
            # ramp needs a sustained burst of instructions (~40+), not cycles,
            # so keep the count high but the moving operand narrow.
            if warmup:
                wsrc = const_pool.tile([P, 256], mybir.dt.bfloat16, tag="wsrc")
                nc.vector.memset(wsrc[:], 0.0)
                wp = warm_ps_pool.tile([P, 128], mybir.dt.float32, tag="wp")
                for _ in range(warmup):
                    nc.tensor.matmul(
                        wp[:], wsrc[:, :P], wsrc[:, :128], start=True, stop=True
                    )

            xg0 = x_pool.tile([P, KPAIRS, 2, mg], f8, tag="xg")

            # Resident fp8 weight tiles, one per k-pair, already centered on
            # host.
            w_tiles = [
                w_pool.tile([P, 2, npc], f8, name=f"W{t}", tag=f"W{t}")
                for t in range(KPAIRS)
            ]
            # interleave weight and first-group activation DMAs so pair t of
            # both lands early, letting mb0's accumulation start ASAP
            for t in range(KPAIRS):
                nc.sync.dma_start(w_tiles[t][:], w8_d[t])
                nc.sync.dma_start(xg0[:, t, :, :], x8_d[t, :, :, 0:mg])

            # scale rows are first needed at the first eviction: keep their
            # 1.4MB out of the pre-mb0 DMA critical path
            nc.sync.dma_start(s32t[:], s_d[:])
            nc.sync.dma_start(cb32t[:], cb_d[:])
            nc.sync.dma_start(rs_t[:], rs_d[:])

            # Main matmul loop: m-groups of `mg` tokens, 128-token m-blocks.
            # The NEXT group's x DMAs are emitted before this group's
            # m-blocks: DMA issue serializes on the Sync queue (~0.7us per
            # DMA_DIRECT2D), so they must enter the queue ahead of this
            # group's out-DMAs to land before the PE needs them.
            xgs = {0: xg0}
            for g in range(ngroups):
                if g + 1 < ngroups:
                    xg_next = x_pool.tile([P, KPAIRS, 2, mg], f8, tag="xg")
                    for t in range(KPAIRS):
                        nc.sync.dma_start(
                            xg_next[:, t, :, :],
                            x8_d[t, :, :, (g + 1) * mg:(g + 2) * mg],
                        )
                    xgs[g + 1] = xg_next
                xg = xgs.pop(g)
                for mb in range(mbs):
                    mbi = g * mbs + mb
                    ps = ps_pool.tile([P, npc], mybir.dt.float32, tag="ps")
                    for t in range(KPAIRS):
                        lhsT = xg[:, t, :, mb * P:(mb + 1) * P]
                        for (n0, nw) in chunks:
                            # start=True marks the whole 2KB PSUM bank (the
                            # zero region) pending-zero, so only a chunk that
                            # begins a bank may issue it; a bank's later
                            # chunks inherit the marking and their first
                            # (start=False) write still overwrites.
                            bank_first = (n0 % 512) == 0
                            nc.tensor.matmul(
                                ps[:, n0:n0 + nw], lhsT,
                                w_tiles[t][:, :, n0:n0 + nw],
                                start=(t == 0 and bank_first),
                                stop=(t == KPAIRS - 1),
                                perf_mode=mybir.MatmulPerfMode.DoubleRow,
                                skip_group_check=(not bank_first),
                            )
                    # eviction (ACT), then dequant on SBUF, chunk-wise so the
                    # ACT/VE/DMA stages pipeline (shrinks the kernel tail):
                    #   out = psum * s + s*(7.5 - z) * rowsum[m]
                    ot = o_pool.tile([P, npc], mybir.dt.float32, tag="ot")
                    m0 = g * mg + mb * P
                    for (n0, nw) in chunks:
                        sl = slice(n0, n0 + nw)
                        nc.scalar.copy(ot[:, sl], ps[:, sl])
                        nc.vector.tensor_tensor(
                            ot[:, sl], ot[:, sl], s32t[:, sl],
                            op=mybir.AluOpType.mult,
                        )
                        nc.vector.scalar_tensor_tensor(
                            ot[:, sl], cb32t[:, sl], rs_t[:, mbi:mbi + 1],
                            ot[:, sl],
                            op0=mybir.AluOpType.mult, op1=mybir.AluOpType.add,
                        )
                        nc.sync.dma_start(out_d[m0:m0 + P, sl], ot[:, sl])

    nc.compile()
    return nc


def prep_inputs(inp, quant_weight, scales, zeros, ncores=NCORES, npc=NPC):
    """Host-side sharding/layout: returns in_maps list for run_bass_kernel_spmd."""
    m = inp.shape[0] * inp.shape[1]
    k = inp.shape[2]

    x = np.asarray(inp, dtype=np.float32).reshape(m, k)
    # x8[t, p, i, tok] = e4m3(x[tok, 256t + 2p + i]): plane i=0 even k (low
    # nibble), i=1 odd k (high nibble), paired per DoubleRow instruction
    x8 = np.ascontiguousarray(
        x.reshape(m, KPAIRS, P, 2).astype(ml_dtypes.float8_e4m3)
        .transpose(1, 2, 3, 0)
    )

    # rowsum of the exact activations, for the center/zero correction term
    rs = x.sum(axis=1, dtype=np.float64).astype(np.float32)  # [m]
    rs_host = np.ascontiguousarray(rs.reshape(m // P, P).T)  # [P, m//P]

    n = quant_weight.shape[0]
    assert n == ncores * npc, (n, ncores, npc)
    qw8 = np.asarray(quant_weight).astype(np.uint8)          # [N, k//2]
    lo = (qw8 & 15).astype(np.float32) - CENTER              # even k
    hi = (qw8 >> 4).astype(np.float32) - CENTER              # odd k
    s_all = np.asarray(scales, dtype=np.float32).reshape(-1)
    z_all = np.asarray(zeros, dtype=np.float32).reshape(-1)
    cb_all = s_all * (CENTER - z_all)

    in_maps = []
    for c in range(ncores):
        sl = slice(c * npc, (c + 1) * npc)
        # w8[t, p, i, n]: centered nibbles, exact in e4m3
        wc = np.stack([lo[sl].T, hi[sl].T], axis=1)          # [k//2, 2, npc]
        wc = np.ascontiguousarray(
            wc.reshape(KPAIRS, P, 2, npc).astype(ml_dtypes.float8_e4m3)
        )
        s_c = np.ascontiguousarray(np.broadcast_to(s_all[sl], (P, npc)))
        cb_c = np.ascontiguousarray(np.broadcast_to(cb_all[sl], (P, npc)))
        in_maps.append(
            {"x8": x8, "w8": wc, "s32": s_c, "cb32": cb_c, "rs": rs_host}
        )
    return in_maps


_NC_CACHE = {}


def _get_nc():
    if "nc" not in _NC_CACHE:
        _NC_CACHE["nc"] = build_nc()
    return _NC_CACHE["nc"]


def kernel(inp, quant_weight, scales, zeros):
    from concourse.bass_utils import run_bass_kernel_spmd

    nc = _get_nc()
    in_maps = prep_inputs(inp, quant_weight, scales, zeros)
    res = run_bass_kernel_spmd(nc, in_maps, list(range(NCORES)))
    out = np.concatenate([res.results[c]["out"] for c in range(NCORES)], axis=1)
    return np.ascontiguousarray(out).reshape(B, S, N)


# revision 24
# speedup vs baseline: 1.0085x; 1.0085x over previous
"""4-bit column-block-quantized linear (ColBlockQuantizedLinear) on 8 Trainium2 NeuronCores.

Reference computation:
    w[n, k] = (nibble(quant_weight)[n, k] - zeros[n]) * scales[n]     n<11008, k<4096
    out[b, s, n] = sum_k inp[b, s, k] * w[n, k]                        inp: [4, 2048, 4096] f32

Strategy (column-parallel, per sharding hint):
  - Shard out_features N=11008 = 8*1376 across 8 cores; replicate inp.
  - fp8 double-pumped matmul (MatmulPerfMode.DoubleRow, 2x the bf16 PE rate):
    both operands are float8e4 (e4m3). Host ships activations rounded to e4m3
    and weights expanded to CENTERED nibbles (q - 7.5), which are exact in
    e4m3 (values +-0.5 .. +-7.5).
  - Centering is the accuracy trick: the fp8 rounding error of the
    activations couples to the matmul weights, so using (q - 7.5) instead of
    raw q (RMS 4.6 vs 8.8) cuts the error ~1.9x. The 7.5 shift is folded
    back exactly at eviction through the f64-accurate host row-sums:
        out = psum * s[n] + s[n]*(7.5 - z[n]) * rowsum[m]
    Measured l2 rel err ~1.7e-2 (vs 3.2e-2 uncentered).
  - K = 4096 = 16 pairs x (2 planes x 128); DoubleRow contracts both planes
    of a pair per instruction: lhsT = x8[128, 2, 128m], moving =
    w8[128, 2, 512n] (fp8 moving free dim max 1024; out = 512 f32 = exactly
    one PSUM bank). start=True marks the whole 2KB PSUM bank pending-zero,
    so only bank-first chunks may issue it.
  - Host pre-expands packed nibbles to centered fp8 weights (no on-chip
    unpack) and ships activations pre-rounded to e4m3: DMA totals ~41MB/core,
    well under the ~360GB/s budget for the ~600us PE-bound runtime.
  - A 120-instruction burst of narrow dummy matmuls warms the PE (HAM
    un-throttle to 8/8) while the DMA stream builds a head start; both the
    burst length and its ~8us duration are load-bearing (shorter warmups
    leave the clock throttled / the PE stalling on the DMA ramp).
  - Eviction is chunk-wise (ACT copy, VE scale, VE rowsum-correct, DMA out
    per 512-col bank) so the tail pipelines.
  - Host concatenates per-core outputs along N.
"""

import sys

for _p in ("/opt/trn_rl_repo", "/opt/pypackages"):
    if _p not in sys.path:
        sys.path.append(_p)

import numpy as np
import ml_dtypes

import concourse.bass as bass
import concourse.mybir as mybir
import concourse.tile as tile
from concourse import bacc

# Problem constants (hardcoded per harness contract)
B, S, K = 4, 2048, 4096
M = B * S                  # 8192 tokens
N = 11008                  # out features
NCORES = 8
NPC = N // NCORES          # per-core out features (1376)
P = 128
KPAIRS = K // (2 * P)      # 16 pairs of k-planes (256 k each)
CENTER = 7.5               # nibble centering; q - 7.5 is exact in e4m3


def _nchunks(npc, cw=256):
    return [(i, min(cw, npc - i)) for i in range(0, npc, cw)]


def build_nc(m=M, npc=NPC, mg=512, warmup=120, cw=512):
    """Build the per-core Bass program. m tokens, npc out cols, mg tokens per
    m-group (DMA granule), cw psum chunk width (512 f32 = one PSUM bank;
    DoubleRow moving free = 2*cw <= 1024)."""
    ngroups = m // mg
    mbs = mg // P              # m-blocks per group
    chunks = _nchunks(npc, cw)
    f8 = mybir.dt.float8e4

    nc = bacc.Bacc("TRN2", target_bir_lowering=False, debug=False)
    x8_d = nc.dram_tensor("x8", [KPAIRS, P, 2, m], f8, kind="ExternalInput")
    w8_d = nc.dram_tensor("w8", [KPAIRS, P, 2, npc], f8, kind="ExternalInput")
    s_d = nc.dram_tensor("s32", [P, npc], mybir.dt.float32, kind="ExternalInput")
    cb_d = nc.dram_tensor("cb32", [P, npc], mybir.dt.float32, kind="ExternalInput")
    rs_d = nc.dram_tensor("rs", [P, m // P], mybir.dt.float32, kind="ExternalInput")
    out_d = nc.dram_tensor("out", [m, npc], mybir.dt.float32, kind="ExternalOutput")

    with tile.TileContext(nc) as tc:
        with (
            tc.tile_pool(name="const", bufs=1) as const_pool,
            tc.tile_pool(name="w", bufs=1) as w_pool,
            tc.tile_pool(name="x", bufs=3) as x_pool,
            tc.tile_pool(name="o", bufs=3) as o_pool,
            tc.tile_pool(name="ps", bufs=2, space="PSUM") as ps_pool,
            tc.tile_pool(name="wps", bufs=1, space="PSUM") as warm_ps_pool,
        ):
            s32t = const_pool.tile([P, npc], mybir.dt.float32, tag="s32t")
            cb32t = const_pool.tile([P, npc], mybir.dt.float32, tag="cb32t")
            rs_t = const_pool.tile([P, m // P], mybir.dt.float32, tag="rs_t")
            # PE warmup: flip the HAM clock gate to 8/8 while DMAs run. The
            # ramp needs a sustained burst of instructions (~40+), not cycles,
            # so keep the count high but the moving operand narrow.
            if warmup:
                wsrc = const_pool.tile([P, 256], mybir.dt.bfloat16, tag="wsrc")
                nc.vector.memset(wsrc[:], 0.0)
                wp = warm_ps_pool.tile([P, 128], mybir.dt.float32, tag="wp")
                for _ in range(warmup):
                    nc.tensor.matmul(
                        wp[:], wsrc[:, :P], wsrc[:, :128], start=True, stop=True
                    )

            xg0 = x_pool.tile([P, KPAIRS, 2, mg], f8, tag="xg")

            # Resident fp8 weight tiles, one per k-pair, already centered on
            # host.
            w_tiles = [
                w_pool.tile([P, 2, npc], f8, name=f"W{t}", tag=f"W{t}")
                for t in range(KPAIRS)
            ]
            # interleave weight and first-group activation DMAs so pair t of
            # both lands early, letting mb0's accumulation start ASAP
            for t in range(KPAIRS):
                nc.sync.dma_start(w_tiles[t][:], w8_d[t])
                nc.sync.dma_start(xg0[:, t, :, :], x8_d[t, :, :, 0:mg])

            # scale rows are first needed at the first eviction: keep their
            # 1.4MB out of the pre-mb0 DMA critical path
            nc.sync.dma_start(s32t[:], s_d[:])
            nc.sync.dma_start(cb32t[:], cb_d[:])
            nc.sync.dma_start(rs_t[:], rs_d[:])

            # Main matmul loop: m-groups of `mg` tokens, 128-token m-blocks.
            # The NEXT group's x DMAs are emitted before this group's
            # m-blocks: DMA issue serializes on the Sync queue (~0.7us per
            # DMA_DIRECT2D), so they must enter the queue ahead of this
            # group's out-DMAs to land before the PE needs them.
            xgs = {0: xg0}
            for g in range(ngroups):
                if g + 1 < ngroups:
                    xg_next = x_pool.tile([P, KPAIRS, 2, mg], f8, tag="xg")
                    for t in range(KPAIRS):
                        nc.sync.dma_start(
                            xg_next[:, t, :, :],
                            x8_d[t, :, :, (g + 1) * mg:(g + 2) * mg],
                        )
                    xgs[g + 1] = xg_next
                xg = xgs.pop(g)
                for mb in range(mbs):
                    mbi = g * mbs + mb
                    ps = ps_pool.tile([P, npc], mybir.dt.float32, tag="ps")
                    for t in range(KPAIRS):
                        lhsT = xg[:, t, :, mb * P:(mb + 1) * P]
                        for (n0, nw) in chunks:
                            # start=True marks the whole 2KB PSUM bank (the
                            # zero region) pending-zero, so only a chunk that
                            # begins a bank may issue it; a bank's later
                            # chunks inherit the marking and their first
                            # (start=False) write still overwrites.
                            bank_first = (n0 % 512) == 0
                            nc.tensor.matmul(
                                ps[:, n0:n0 + nw], lhsT,
                                w_tiles[t][:, :, n0:n0 + nw],
                                start=(t == 0 and bank_first),
                                stop=(t == KPAIRS - 1),
                                perf_mode=mybir.MatmulPerfMode.DoubleRow,
                                skip_group_check=(not bank_first),
                            )
                    # eviction (ACT), then dequant on SBUF, chunk-wise so the
                    # ACT/VE/DMA stages pipeline (shrinks the kernel tail):
                    #   out = psum * s + s*(7.5 - z) * rowsum[m]
                    ot = o_pool.tile([P, npc], mybir.dt.float32, tag="ot")
                    m0 = g * mg + mb * P
                    for (n0, nw) in chunks:
                        sl = slice(n0, n0 + nw)
                        nc.scalar.copy(ot[:, sl], ps[:, sl])
                        nc.vector.tensor_tensor(
                            ot[:, sl], ot[:, sl], s32t[:, sl],
                            op=mybir.AluOpType.mult,
                        )
                        nc.vector.scalar_tensor_tensor(
                            ot[:, sl], cb32t[:, sl], rs_t[:, mbi:mbi + 1],
                            ot[:, sl],
                            op0=mybir.AluOpType.mult, op1=mybir.AluOpType.add,
                        )
                        nc.sync.dma_start(out_d[m0:m0 + P, sl], ot[:, sl])

    nc.compile()
    return nc


def prep_inputs(inp, quant_weight, scales, zeros, ncores=NCORES, npc=NPC):
    """Host-side sharding/layout: returns in_maps list for run_bass_kernel_spmd."""
    m = inp.shape[0] * inp.shape[1]
    k = inp.shape[2]

    x = np.asarray(inp, dtype=np.float32).reshape(m, k)
    # x8[t, p, i, tok] = e4m3(x[tok, 256t + 2p + i]): plane i=0 even k (low
    # nibble), i=1 odd k (high nibble), paired per DoubleRow instruction
    x8 = np.ascontiguousarray(
        x.reshape(m, KPAIRS, P, 2).astype(ml_dtypes.float8_e4m3)
        .transpose(1, 2, 3, 0)
    )

    # rowsum of the exact activations, for the center/zero correction term
    rs = x.sum(axis=1, dtype=np.float64).astype(np.float32)  # [m]
    rs_host = np.ascontiguousarray(rs.reshape(m // P, P).T)  # [P, m//P]

    n = quant_weight.shape[0]
    assert n == ncores * npc, (n, ncores, npc)
    qw8 = np.asarray(quant_weight).astype(np.uint8)          # [N, k//2]
    lo = (qw8 & 15).astype(np.float32) - CENTER              # even k
    hi = (qw8 >> 4).astype(np.float32) - CENTER              # odd k
    s_all = np.asarray(scales, dtype=np.float32).reshape(-1)
    z_all = np.asarray(zeros, dtype=np.float32).reshape(-1)
    cb_all = s_all * (CENTER - z_all)

    in_maps = []
    for c in range(ncores):
        sl = slice(c * npc, (c + 1) * npc)
        # w8[t, p, i, n]: centered nibbles, exact in e4m3
        wc = np.stack([lo[sl].T, hi[sl].T], axis=1)          # [k//2, 2, npc]
        wc = np.ascontiguousarray(
            wc.reshape(KPAIRS, P, 2, npc).astype(ml_dtypes.float8_e4m3)
        )
        s_c = np.ascontiguousarray(np.broadcast_to(s_all[sl], (P, npc)))
        cb_c = np.ascontiguousarray(np.broadcast_to(cb_all[sl], (P, npc)))
        in_maps.append(
            {"x8": x8, "w8": wc, "s32": s_c, "cb32": cb_c, "rs": rs_host}
        )
    return in_maps


_NC_CACHE = {}


def _get_nc():
    if "nc" not in _NC_CACHE:
        _NC_CACHE["nc"] = build_nc()
    return _NC_CACHE["nc"]


def kernel(inp, quant_weight, scales, zeros):
    from concourse.bass_utils import run_bass_kernel_spmd

    nc = _get_nc()
    in_maps = prep_inputs(inp, quant_weight, scales, zeros)
    res = run_bass_kernel_spmd(nc, in_maps, list(range(NCORES)))
    out = np.concatenate([res.results[c]["out"] for c in range(NCORES)], axis=1)
    return np.ascontiguousarray(out).reshape(B, S, N)
